# revision 26
# baseline (speedup 1.0000x reference)
"""Multi-head attention (B=2, T=2048, H=1024, 16 heads) on 8 trn2 cores.

Sharding: data-parallel over batch (2) x tensor-parallel over head groups
(4 heads/core).  Each core computes qkv projections for its 4 heads,
attention, and a partial out-projection; the host sums the 4 partials per
batch and adds b_out.

Key structure (v1 rewrite):
- Compressed KV: the key mask is known per batch, so the host gathers only
  the unmasked tokens into x_kv (padded to a 128 multiple).  Scores, the
  softmax exp, and AV run over ~half the keys.  Padded key slots get an
  exp bias of -1e9 (exp -> 0) so they contribute nothing.
- All matmul operands are bf16 (PSUM accumulation stays f32).  The host
  pre-converts x and the weight slices.
- xT tiles are produced by XBAR DMA-transpose loads (2-byte dtype), not PE
  transposes.  V is transposed head-by-head with SBUF->SBUF DMA-transpose
  into the ones-augmented vp layout used by the AV matmul.
- Scores for the two heads of a pair run as two concurrent row-tiled
  matmuls (K=64 each, PE row groups 0-63 / 64-127), one [128,1024] PSUM
  tile, one fused mask+scale+exp ACT op for both heads.
- AV uses ones-augmented V so the softmax denominator accumulates in PSUM
  row 64.  Normalization: gpsimd copies the two denominator rows to SBUF,
  one DVE reciprocal_approx_fast over [2,512], PE broadcast via ones
  matmul, gpsimd multiply fused with the PSUM->SBUF move of the numerator.
- The out-projection (K=128 over head pairs, accumulated over both pairs)
  is interleaved per 512-query block inside pair-1 attention so the PE
  fills the slack of the ACT-bound softmax loop.  Output partials are
  stored bf16; the host sums in f32 and adds b_out.
"""

import sys

sys.path.insert(0, "/opt/trn_rl_repo")

import numpy as np

B, T, H = 2, 2048, 1024
NH, DK = 16, 64
HPC = 4           # heads per core
NCORES = 8

_CACHE = {}


def _build(t_kv):
    import concourse.bacc as bacc
    import concourse.mybir as mybir
    import concourse.tile as tile
    from concourse.masks import make_identity

    f32 = mybir.dt.float32
    f32r = mybir.dt.float32r
    bf16 = mybir.dt.bfloat16
    AF = mybir.ActivationFunctionType
    ALU = mybir.AluOpType

    n_kt = t_kv // 128
    KC = H // 128     # 8 contraction chunks

    nc = bacc.Bacc("TRN2", target_bir_lowering=False, debug=False)

    # x_kv is pre-chunked on host to [KC, t_kv, 128] so each h-chunk is a
    # contiguous DRAM region (the XBAR DMA-transpose mishandles strided APs).
    # x (the larger Q-side input) is transposed by the PE instead, which is
    # idle during startup, so the single XBAR queue only carries x_kv.
    x_d = nc.dram_tensor("x", [T, H], bf16, kind="ExternalInput")
    xkv_d = nc.dram_tensor("x_kv", [H // 128, t_kv, 128], bf16, kind="ExternalInput")
    wq_d = nc.dram_tensor("w_q", [H, 256], bf16, kind="ExternalInput")
    wk_d = nc.dram_tensor("w_k", [H, 256], bf16, kind="ExternalInput")
    wv_d = nc.dram_tensor("w_v", [H, 256], bf16, kind="ExternalInput")
    wout_d = nc.dram_tensor("w_out", [HPC * DK, H], bf16, kind="ExternalInput")
    bq_d = nc.dram_tensor("bqkvt", [128, 6], f32, kind="ExternalInput")
    padb_d = nc.dram_tensor("padbias", [128, n_kt], f32, kind="ExternalInput")
    out_d = nc.dram_tensor("out_partial", [T, H], bf16, kind="ExternalOutput")

    with tile.TileContext(nc) as tc:
        with (
            tc.tile_pool(name="persist", bufs=1) as pp,
            tc.tile_pool(name="small", bufs=1) as sp,
            tc.tile_pool(name="expp", bufs=6) as ep,
            tc.tile_pool(name="ostage", bufs=4) as osp,
            tc.tile_pool(name="normp", bufs=2) as dn,
            tc.tile_pool(name="psum", bufs=1, space="PSUM") as psp,
        ):
            bq = sp.tile([128, 6], f32, tag="bq", name="bq")
            nc.gpsimd.dma_start(out=bq, in_=bq_d[:, :])
            padb = sp.tile([128, n_kt], f32, tag="padb", name="padb")
            nc.gpsimd.dma_start(out=padb, in_=padb_d[:, :])
            wout = sp.tile([128, 2 * H], bf16, tag="wout", name="wout")
            # transposed x tiles via XBAR DMA transpose, one full tile per
            # h-chunk (the XBAR ucode only handles full-tile outputs) so the
            # first projection matmuls start after ~1 chunk lands.
            # K/V inputs land first; xqT on a separate queue.
            ident = sp.tile([128, 128], bf16, tag="ident", name="ident")
            make_identity(nc, ident)
            xqT = pp.tile([128, KC * T], bf16, tag="xqT", name="xqT")
            xqT_v = xqT.rearrange("p (c t) -> p c t", t=T)
            xqT_c = [xqT_v[:, c, :] for c in range(KC)]
            xkvT_c = [
                pp.tile([128, t_kv], bf16, tag=f"xkvT{c}", name=f"xkvT{c}")
                for c in range(KC)
            ]
            wq = pp.tile([128, KC * 256], bf16, tag="wq", name="wq")
            wk = pp.tile([128, KC * 256], bf16, tag="wk", name="wk")
            wv = pp.tile([128, KC * 256], bf16, tag="wv", name="wv")
            for c in range(KC):
                nc.sync.dma_start_transpose(xkvT_c[c], xkv_d[c, :, :])
                nc.gpsimd.dma_start(
                    out=wk[:, c * 256 : (c + 1) * 256],
                    in_=wk_d[c * 128 : (c + 1) * 128, :],
                )
                nc.gpsimd.dma_start(
                    out=wv[:, c * 256 : (c + 1) * 256],
                    in_=wv_d[c * 128 : (c + 1) * 128, :],
                )
            for c in range(KC):
                nc.scalar.dma_start(
                    out=wq[:, c * 256 : (c + 1) * 256],
                    in_=wq_d[c * 128 : (c + 1) * 128, :],
                )
            # PE-transpose x into xqT: per 128-token tile, transpose the 8
            # h-chunks through PSUM (bf16, 1 cycle/row) in two 4-chunk groups,
            # each moved to SBUF by one strided DVE copy.
            for tt in range(T // 128):
                xtile = dn.tile([128, H], bf16, tag="xtile", name="xtile")
                for half in range(2):
                    nc.scalar.dma_start(
                        out=xtile[:, half * 512 : (half + 1) * 512],
                        in_=x_d[
                            tt * 128 : (tt + 1) * 128,
                            half * 512 : (half + 1) * 512,
                        ],
                    )
                for g in range(2):
                    pt = psp.tile([128, 512], bf16, tag="big", bufs=2, name="pt")
                    for j in range(4):
                        c = g * 4 + j
                        nc.tensor.transpose(
                            pt[:, j * 128 : (j + 1) * 128],
                            xtile[:, c * 128 : (c + 1) * 128],
                            ident,
                        )
                    nc.vector.tensor_copy(
                        xqT_v[
                            :, g * 4 : (g + 1) * 4, tt * 128 : (tt + 1) * 128
                        ],
                        pt.rearrange("p (j t) -> p j t", t=128),
                    )
            for p in range(2):
                nc.scalar.dma_start(
                    out=wout[:, p * H : (p + 1) * H],
                    in_=wout_d[p * 128 : (p + 1) * 128, :],
                )

            qT = [pp.tile([128, T], bf16, tag=f"qT{p}", name=f"qT{p}") for p in range(2)]
            kT = [pp.tile([128, t_kv], bf16, tag=f"kT{p}", name=f"kT{p}") for p in range(2)]
            vT = [pp.tile([128, t_kv], bf16, tag=f"vT{p}", name=f"vT{p}") for p in range(2)]
            vp = [
                pp.tile([128, n_kt * 65], bf16, tag=f"vp{h}", name=f"vp{h}")
                for h in range(HPC)
            ]
            attn = [
                pp.tile([128, T], bf16, tag=f"attn{p}", name=f"attn{p}")
                for p in range(2)
            ]
            for h in range(HPC):
                vpv = vp[h].rearrange("p (k c) -> p k c", c=65)
                nc.gpsimd.memset(vpv[:, :, 64:65], 1.0)

            def project(w_sb, mt, bias_col, dst, t_len, xT_tiles):
                # dst[128, t_len] = (W_mt.T @ xT) + bias, bf16
                for nb0 in range(0, t_len, 512):
                    w512 = min(512, t_len - nb0)
                    ps = psp.tile([128, 512], f32, tag="misc", bufs=2, name="ps")
                    for c in range(KC):
                        nc.tensor.matmul(
                            ps[:, 0:w512],
                            w_sb[:, c * 256 + mt * 128 : c * 256 + (mt + 1) * 128],
                            xT_tiles[c][:, nb0 : nb0 + w512],
                            start=(c == 0),
                            stop=(c == KC - 1),
                        )
                    nc.vector.tensor_scalar_add(
                        dst[:, nb0 : nb0 + w512],
                        ps[:, 0:w512],
                        bq[:, bias_col : bias_col + 1],
                    )

            def build_vp(pair):
                # XBAR DMA-transpose needs a contiguous output AP on HW;
                # stage contiguously, then strided-copy on gpsimd.
                for lh in range(2):
                    h = pair * 2 + lh
                    vst = dn.tile([128, n_kt * 64], bf16, tag="vst", name="vst")
                    nc.sync.dma_start_transpose(
                        vst.rearrange("p (k c) -> p k c", c=64),
                        vT[pair][lh * 64 : (lh + 1) * 64, :],
                    )
                    vpv = vp[h].rearrange("p (k c) -> p k c", c=65)
                    nc.gpsimd.tensor_copy(
                        vpv[:, :, 0:64], vst.rearrange("p (k c) -> p k c", c=64)
                    )

            def outproj_nb(nb):
                for j in range(4):
                    mt = nb * 4 + j
                    for ob in range(2):
                        po = psp.tile([128, 512], f32, tag="misc", bufs=2, name="po")
                        for p in range(2):
                            nc.tensor.matmul(
                                po,
                                attn[p][:, mt * 128 : (mt + 1) * 128],
                                wout[:, p * H + ob * 512 : p * H + ob * 512 + 512],
                                start=(p == 0),
                                stop=(p == 1),
                            )
                        ot = osp.tile([128, 512], bf16, tag="ot", name="ot")
                        nc.vector.tensor_copy(ot, po)
                        nc.scalar.dma_start(
                            out=out_d[
                                mt * 128 : (mt + 1) * 128,
                                ob * 512 : ob * 512 + 512,
                            ],
                            in_=ot,
                        )

            def attn_pair(pair, interleave_outproj):
                p = pair
                for nb in range(4):
                    accs = [
                        psp.tile([65, 512], f32, tag="acc", bufs=2, name="acc")
                        for lh in range(2)
                    ]
                    for kt in range(n_kt):
                        ss = psp.tile([128, 1024], f32, tag="big", bufs=2, name="ss")
                        for lh in range(2):
                            r0 = lh * 64
                            nc.tensor.matmul(
                                ss[:, lh * 512 : (lh + 1) * 512],
                                kT[p][r0 : r0 + 64, kt * 128 : (kt + 1) * 128],
                                qT[p][r0 : r0 + 64, nb * 512 : nb * 512 + 512],
                                start=True,
                                stop=True,
                            )
                        ex = ep.tile([128, 1024], bf16, tag="ex", name="ex")
                        nc.scalar.activation(
                            ex, ss, AF.Exp, bias=padb[:, kt : kt + 1], scale=0.125
                        )
                        for lh in range(2):
                            nc.tensor.matmul(
                                accs[lh],
                                vp[2 * p + lh][:, kt * 65 : kt * 65 + 65],
                                ex[:, lh * 512 : (lh + 1) * 512],
                                start=(kt == 0),
                                stop=(kt == n_kt - 1),
                            )
                    # normalization: recip of denom row (DVE, PSUM in),
                    # partition-broadcast on gpsimd, multiply fused with the
                    # PSUM->SBUF move of the numerator (DVE).
                    for lh in range(2):
                        r0 = lh * 64
                        dcp = dn.tile([1, 512], f32, tag=f"dcp{lh}", name="dcp")
                        nc.vector.tensor_copy(dcp, accs[lh][64:65, :])
                        rst = dn.tile([1, 512], f32, tag=f"rst{lh}", name="rst")
                        nc.vector.reciprocal_approx_fast(rst, dcp)
                        recb = dn.tile([64, 512], f32, tag=f"recb{lh}", name="recb")
                        nc.gpsimd.partition_broadcast(recb, rst)
                        nc.vector.tensor_tensor(
                            out=attn[p][r0 : r0 + 64, nb * 512 : nb * 512 + 512],
                            in0=accs[lh][0:64, :],
                            in1=recb,
                            op=ALU.mult,
                        )
                    if interleave_outproj:
                        outproj_nb(nb)

            # ---- schedule ----
            # bias cols: 0,1 -> k01,k23 ; 2,3 -> v01,v23 ; 4,5 -> q01,q23
            project(wk, 0, 0, kT[0], t_kv, xkvT_c)
            project(wv, 0, 2, vT[0], t_kv, xkvT_c)
            build_vp(0)
            project(wq, 0, 4, qT[0], T, xqT_c)
            project(wk, 1, 1, kT[1], t_kv, xkvT_c)
            project(wv, 1, 3, vT[1], t_kv, xkvT_c)
            build_vp(1)
            project(wq, 1, 5, qT[1], T, xqT_c)
            attn_pair(0, False)
            attn_pair(1, True)

    nc.compile()
    return nc


def _get_nc(t_kv):
    key = f"nc{t_kv}"
    if key not in _CACHE:
        _CACHE[key] = _build(t_kv)
    return _CACHE[key]


def _prep_in_maps(x, mask, W_qkv, b_qkv, W_out, t_kv, idxs):
    import ml_dtypes

    bf16 = ml_dtypes.bfloat16
    n_kt = t_kv // 128
    in_maps = []
    for c in range(NCORES):
        b = c // 4
        h0 = (c % 4) * HPC
        idx = idxs[b]
        n_real = len(idx)
        xkv = np.zeros((t_kv, H), dtype=np.float32)
        xkv[:n_real] = x[b][idx]

        sl_q = slice(0 * H + h0 * DK, 0 * H + (h0 + HPC) * DK)
        sl_k = slice(1 * H + h0 * DK, 1 * H + (h0 + HPC) * DK)
        sl_v = slice(2 * H + h0 * DK, 2 * H + (h0 + HPC) * DK)
        bqt = np.zeros((128, 6), dtype=np.float32)
        bqt[:, 0] = b_qkv[sl_k][:128]
        bqt[:, 1] = b_qkv[sl_k][128:]
        bqt[:, 2] = b_qkv[sl_v][:128]
        bqt[:, 3] = b_qkv[sl_v][128:]
        bqt[:, 4] = b_qkv[sl_q][:128]
        bqt[:, 5] = b_qkv[sl_q][128:]

        padb = np.zeros((128, n_kt), dtype=np.float32)
        for j in range(n_real, t_kv):
            padb[j % 128, j // 128] = -1e9

        in_maps.append(
            {
                "x": np.ascontiguousarray(x[b]).astype(bf16),
                "x_kv": np.ascontiguousarray(
                    xkv.reshape(t_kv, H // 128, 128).swapaxes(0, 1)
                ).astype(bf16),
                "w_q": np.ascontiguousarray(W_qkv[:, sl_q]).astype(bf16),
                "w_k": np.ascontiguousarray(W_qkv[:, sl_k]).astype(bf16),
                "w_v": np.ascontiguousarray(W_qkv[:, sl_v]).astype(bf16),
                "w_out": np.ascontiguousarray(
                    W_out[h0 * DK : (h0 + HPC) * DK, :]
                ).astype(bf16),
                "bqkvt": bqt,
                "padbias": padb,
            }
        )
    return in_maps


def _combine(partials, b_out):
    out = np.empty((B, T, H), dtype=np.float32)
    for b in range(B):
        acc = partials[4 * b].astype(np.float32)
        for i in range(1, 4):
            acc = acc + partials[4 * b + i].astype(np.float32)
        out[b] = acc + b_out[None, :]
    return out


def _plan(mask):
    idxs = [np.nonzero(np.asarray(mask)[b, 0, 0, :])[0] for b in range(B)]
    n_max = max(1, max(len(i) for i in idxs))
    t_kv = min(T, ((n_max + 127) // 128) * 128)
    return t_kv, idxs


def kernel(x, mask, W_qkv, b_qkv, W_out, b_out):
    x = np.asarray(x, dtype=np.float32)
    mask = np.asarray(mask)
    W_qkv = np.asarray(W_qkv, dtype=np.float32)
    b_qkv = np.asarray(b_qkv, dtype=np.float32)
    W_out = np.asarray(W_out, dtype=np.float32)
    b_out = np.asarray(b_out, dtype=np.float32)

    t_kv, idxs = _plan(mask)
    nc = _get_nc(t_kv)
    in_maps = _prep_in_maps(x, mask, W_qkv, b_qkv, W_out, t_kv, idxs)

    from concourse.bass_utils import run_bass_kernel_spmd

    res = run_bass_kernel_spmd(nc, in_maps, list(range(NCORES)))
    partials = [res.results[c]["out_partial"] for c in range(NCORES)]
    return _combine(partials, b_out)
